# revision 27
# baseline (speedup 1.0000x reference)
"""BEV voxel-pooling (segment_reduce) kernel for 8 Trainium2 NeuronCores.

Strategy
--------
The BEV pooling is a segment-sum of 346k points x 64 channels into ~12k
occupied bins (mean run length ~29).  The whole reduction runs on the
DVE with channels on the partition axis:

Host (numpy, driven only by the small geometry inputs):
  * compute each point's BEV rank (bin id) exactly as the reference
  * segments = runs of equal rank in the sorted point stream
  * split each segment into a BODY (groups of 8 points; a remainder of
    5-7 points becomes one extra zero-padded group) and a TAIL
    (remainder 1-4 points, padded to a power-of-2 class 2/4)
  * bucket bodies by padded group count; deal every class round-robin
    across 16 bins (8 cores x 2 partition-halves) so all cores get an
    identical layout and balanced points
  * per core, build a [128, F] fp16 table: partitions 0-63 hold the 64
    channels of the core's "half A" point stream, 64-127 of "half B".
    Within each body DMA chunk (and each tail class region) points are
    stored in PLANE layout: plane k holds the k-th point of every
    group, so every tree round below is a flat contiguous add (2x DVE)

Device (per core, one SPMD Bass/Tile program, DVE only):
  * DMA the table in ramped chunks, all on the sync queue (a single
    FIFO keeps all 16 DMA engines busy and avoids cross-queue races)
  * per body chunk: 3 flat tensor_tensor adds (halves of planes) fold
    8 points -> 1 group partial (fp16, 2x DVE mode)
  * per body class, one strided tensor_reduce (inner = padded group
    count) sums each segment's partials into the f32 output
  * tail classes are power-of-2 plane trees; the last round writes the
    f32 output columns directly
  * the f32 output [128, S] is DMA'd out in two pieces, the first
    overlapped with the last chunks' compute

Host gather: map each (core, half, class, slot) to its (sample, rank)
and accumulate all pieces into the (B, 40000, 64) grid with np.add.at
(a segment may contribute a body piece and a tail piece); reshape to
the reference layout (B, C, X, Y).
"""
import sys
sys.path.insert(0, '/opt/trn_rl_repo')

import numpy as np

# ---------------- problem constants (hardcoded per spec) ----------------
B, N, C = 2, 6, 64
H_IMG, W_IMG = 256, 704
DS = 16
DSH, DSW = H_IMG // DS, W_IMG // DS          # 16, 44
D0, D1 = 4, 45                                # depth bins -> D = 41
X, Y, Z = 200, 200, 1
NBINS = X * Y * Z                             # 40000
NP_SAMPLE = N * (D1 - D0) * DSH * DSW         # 173184
NCORES = 8
NBIN = NCORES * 2                             # (core, half) bins
G = 8                                         # body group size
BODY_CLASSES = (1, 2, 3, 4, 5, 6, 8, 10, 12, 16, 20, 24, 32, 48)
TAIL_CLASSES = (2, 4)                         # t>=5 folds into body q+=1
BODY_CUTS = (0.0, 0.025, 0.07, 0.15, 0.27, 0.43, 0.62, 0.81, 0.95, 1.0)

_compiled = {}


# ---------------- host geometry (matches reference numerics) ----------------
def _compute_ranks(frustum, post_trans, post_rots, intrinsics, extrinsics,
                   bev_res, bev_start_pos):
    frustum = np.asarray(frustum, np.float32)
    post_trans = np.asarray(post_trans, np.float32)
    post_rots = np.asarray(post_rots, np.float32)
    intrinsics = np.asarray(intrinsics, np.float32)
    extrinsics = np.asarray(extrinsics, np.float32)
    bev_res = np.asarray(bev_res, np.float32)
    bev_start_pos = np.asarray(bev_start_pos, np.float32)

    ext_inv = np.linalg.inv(extrinsics.astype(np.float64)).astype(np.float32)
    rot = ext_inv[..., :3, :3]
    trans = ext_inv[..., :3, 3]
    pts = frustum[None, None] - post_trans[:, :, None, None, None, :]
    pr_inv = np.linalg.inv(post_rots.astype(np.float64)).astype(np.float32)
    pts = np.einsum('bnij,bndhwj->bndhwi', pr_inv, pts).astype(np.float32)
    pts = np.concatenate([pts[..., :2] * pts[..., 2:3], pts[..., 2:3]], axis=-1)
    comb = (rot @ np.linalg.inv(intrinsics.astype(np.float64)).astype(np.float32)
            ).astype(np.float32)
    pts = np.einsum('bnij,bndhwj->bndhwi', comb, pts).astype(np.float32)
    geom = pts + trans[:, :, None, None, None, :]

    coords = (geom - (bev_start_pos - bev_res / 2.0)) / bev_res
    ci = coords.reshape(B, -1, 3).astype(np.int32)
    mask = ((ci[..., 0] >= 0) & (ci[..., 0] < X) &
            (ci[..., 1] >= 0) & (ci[..., 1] < Y) &
            (ci[..., 2] >= 0) & (ci[..., 2] < Z))
    rank = ci[..., 0] * (Y * Z) + ci[..., 1] * Z + ci[..., 2]
    return rank, mask


# ---------------- host planning ----------------
def _plan(rank, mask):
    bcl = np.asarray(BODY_CLASSES)
    tcl = np.asarray(TAIL_CLASSES)
    body_by_class = [[] for _ in BODY_CLASSES]
    tail_by_class = [[] for _ in TAIL_CLASSES]
    orders = []
    for b in range(B):
        m = mask[b]
        valid_idx = np.nonzero(m)[0]
        order = valid_idx[np.argsort(rank[b][valid_idx], kind='stable')]
        orders.append(order)
        rs = rank[b][order]
        new = np.r_[True, rs[1:] != rs[:-1]]
        starts = np.nonzero(new)[0]
        lens = np.diff(np.r_[starts, len(rs)])
        ranks = rs[starts]
        q, t = lens // G, lens % G
        # t >= 5 becomes an extra 8-padded body group (a second body piece,
        # class q=1) -- cheaper than a dedicated tail class
        big_t = t >= 5
        qi = np.searchsorted(bcl, q)
        ti = np.searchsorted(tcl, t)
        for j in range(len(starts)):
            if q[j] > 0:
                body_by_class[qi[j]].append(
                    (b, int(ranks[j]), int(starts[j]), int(q[j]) * G))
            if t[j] > 0:
                if big_t[j]:
                    body_by_class[0].append(
                        (b, int(ranks[j]), int(starts[j] + q[j] * G),
                         int(t[j])))
                else:
                    tail_by_class[ti[j]].append(
                        (b, int(ranks[j]), int(starts[j] + q[j] * G),
                         int(t[j])))

    def deal(by_class, rot):
        bins = [[[] for _ in by_class] for _ in range(NBIN)]
        S = []
        for c, segs in enumerate(by_class):
            segs.sort(key=lambda x: -x[3])
            for i, sg in enumerate(segs):
                bins[(rot + i) % NBIN][c].append(sg)
            S.append((len(segs) + NBIN - 1) // NBIN)
            rot = (rot + len(segs)) % NBIN
        return bins, S, rot

    bbins, Sb, rot = deal(body_by_class, 0)
    tbins, St, _ = deal(tail_by_class, rot)

    Fb = int(sum(Sb[c] * BODY_CLASSES[c] * G for c in range(len(BODY_CLASSES))))
    Ft = int(sum(St[c] * TAIL_CLASSES[c] for c in range(len(TAIL_CLASSES))))
    Stot = int(sum(Sb) + sum(St))

    # body chunk cuts at group granularity
    ngroups = Fb // G
    cuts = [int(round(f * ngroups)) for f in BODY_CUTS]
    cuts[-1] = ngroups
    return orders, bbins, Sb, tbins, St, Fb, Ft, Stot, tuple(cuts)


def _build_core_inputs(core, plan, feats16):
    orders, bbins, Sb, tbins, St, Fb, Ft, Stot, cuts = plan
    F = Fb + Ft
    ngroups = Fb // G
    table = np.zeros((128, F), np.float16)
    out_sample = np.zeros((2, Stot), np.int32)
    out_rank = np.full((2, Stot), -1, np.int32)
    for h in range(2):
        # group-major point index grid [ngroups, 8] (-1 = pad)
        gp = np.full((ngroups, G), -1, np.int64)
        g0 = 0
        slot = 0
        for c, p in enumerate(BODY_CLASSES):
            for s, (b, r, st, ln) in enumerate(bbins[2 * core + h][c]):
                rows = orders[b][st:st + ln] + b * NP_SAMPLE
                gbase = g0 + s * p
                full = np.full(((ln + G - 1) // G) * G, -1, np.int64)
                full[:ln] = rows
                gp[gbase:gbase + len(full) // G] = full.reshape(-1, G)
                out_sample[h, slot + s] = b
                out_rank[h, slot + s] = r
            g0 += Sb[c] * p
            slot += Sb[c]
        # plane layout per chunk
        idx = np.empty(F, np.int64)
        for i in range(len(cuts) - 1):
            ga, gb = cuts[i], cuts[i + 1]
            idx[ga * G:gb * G] = gp[ga:gb].T.reshape(-1)
        # tail region: per class, [S_t, t] -> planes [t, S_t]
        off = Fb
        for c, t in enumerate(TAIL_CLASSES):
            tp = np.full((St[c], t), -1, np.int64)
            for s, (b, r, st, ln) in enumerate(tbins[2 * core + h][c]):
                tp[s, :ln] = orders[b][st:st + ln] + b * NP_SAMPLE
                out_sample[h, slot + s] = b
                out_rank[h, slot + s] = r
            idx[off:off + St[c] * t] = tp.T.reshape(-1)
            off += St[c] * t
            slot += St[c]
        sel = idx >= 0
        tah = np.zeros((F, C), np.float16)
        tah[sel] = feats16[idx[sel]]
        table[h * C:(h + 1) * C] = tah.T
    return {"table": table}, (out_sample, out_rank)


# ---------------- device program ----------------
def _build_kernel(Sb, St, Fb, Ft, Stot, cuts):
    import concourse.bass as bass
    import concourse.bacc as bacc
    import concourse.mybir as mybir
    import concourse.tile as tile
    from contextlib import ExitStack

    F16 = mybir.dt.float16
    F32 = mybir.dt.float32
    ADD = mybir.AluOpType.add
    nc = bacc.Bacc()
    F = Fb + Ft
    S1 = Sb[0]                                 # p=1 class: r3 IS the result
    Sob = Stot - S1                            # f32 output columns
    table = nc.dram_tensor("table", [128, F], F16, kind="ExternalInput")
    outd = nc.dram_tensor("out", [128, Sob], F32, kind="ExternalOutput")
    out16 = nc.dram_tensor("out16", [128, max(S1, 1)], F16,
                           kind="ExternalOutput")

    nb = len(cuts) - 1
    # body class spans (classes >= 2; class p=1 ships straight from r3)
    bspan = []
    goff = S1
    soff = 0
    for c in range(1, len(BODY_CLASSES)):
        p = BODY_CLASSES[c]
        bspan.append((goff, Sb[c], p, soff))
        goff += Sb[c] * p
        soff += Sb[c]
    smid = soff                                # first tail output slot

    with tile.TileContext(nc) as tc, ExitStack() as ctx:
        pool = ctx.enter_context(tc.tile_pool(name="main", bufs=1))
        tbl = pool.tile([128, F], F16)
        r1 = pool.tile([128, Fb // 2], F16)
        r2 = pool.tile([128, Fb // 4], F16)
        r3 = pool.tile([128, Fb // 8], F16)
        t4h = pool.tile([128, max(St[1] * 2, 1)], F16)
        ob = pool.tile([128, Sob], F32)
        # chunks whose r1 runs on gpsimd (private buffer: no Vector writes,
        # so no cross-engine tile serialization)
        goff_chunks = (5, 6)
        r1g_n = sum((cuts[i + 1] - cuts[i]) * 4 for i in goff_chunks)
        r1g = pool.tile([128, max(r1g_n, 1)], F16)

        with nc.allow_low_precision(reason="fp16 tree partials; quantization "
                                    "error well under the 2e-2 gate"):
            # input DMAs all on the sync queue (keeps all 16 DMA engines on
            # one FIFO; chunk 0 small so compute starts early, tail region
            # second so its reduces fill the early pipeline)
            nc.sync.dma_start(tbl[:, 0:cuts[1] * G], table[:, 0:cuts[1] * G])
            for i in range(1, nb):
                a, b = cuts[i] * G, cuts[i + 1] * G
                nc.sync.dma_start(tbl[:, a:b], table[:, a:b])
                if i == nb - 3 and Ft:
                    nc.sync.dma_start(tbl[:, Fb:F], table[:, Fb:F])

            # gpsimd r1 for the offloaded chunks, issued up front so the
            # Pool engine starts as soon as each chunk lands
            r1g_off = {}
            _o = 0
            for i in goff_chunks:
                ga, gb = cuts[i], cuts[i + 1]
                g = gb - ga
                a = ga * G
                nc.gpsimd.tensor_tensor(
                    r1g[:, _o:_o + 4 * g],
                    tbl[:, a:a + 4 * g], tbl[:, a + 4 * g:a + 8 * g], ADD)
                r1g_off[i] = _o
                _o += 4 * g

            def tree(i):
                ga, gb = cuts[i], cuts[i + 1]
                g = gb - ga
                a = ga * G
                if i in r1g_off:
                    o = r1g_off[i]
                    nc.vector.tensor_tensor(
                        r2[:, ga * 2:gb * 2],
                        r1g[:, o:o + 2 * g], r1g[:, o + 2 * g:o + 4 * g], ADD)
                else:
                    nc.vector.tensor_tensor(
                        r1[:, ga * 4:gb * 4],
                        tbl[:, a:a + 4 * g], tbl[:, a + 4 * g:a + 8 * g], ADD)
                    nc.vector.tensor_tensor(
                        r2[:, ga * 2:gb * 2],
                        r1[:, ga * 4:ga * 4 + 2 * g],
                        r1[:, ga * 4 + 2 * g:gb * 4], ADD)
                nc.vector.tensor_tensor(
                    r3[:, ga:gb],
                    r2[:, ga * 2:ga * 2 + g], r2[:, ga * 2 + g:gb * 2], ADD)

            def body_l2(g0, nslot, p, s0):
                src = r3[:, g0:g0 + nslot * p].rearrange("p (s q) -> p s q", q=p)
                nc.vector.tensor_reduce(ob[:, s0:s0 + nslot], src,
                                        axis=mybir.AxisListType.X, op=ADD)

            def tail_trees():
                # class order in table/slots: t=2, t=4 (plane layout)
                n2, n4 = St[0], St[1]
                o2 = Fb
                o4 = o2 + n2 * 2
                s4 = smid + n2
                if n4:
                    nc.vector.tensor_tensor(
                        t4h[:, :n4 * 2],
                        tbl[:, o4:o4 + 2 * n4], tbl[:, o4 + 2 * n4:o4 + 4 * n4],
                        ADD)
                    nc.vector.tensor_tensor(
                        ob[:, s4:s4 + n4],
                        t4h[:, :n4], t4h[:, n4:2 * n4], ADD)
                if n2:
                    nc.vector.tensor_tensor(
                        ob[:, smid:smid + n2],
                        tbl[:, o2:o2 + n2], tbl[:, o2 + n2:o2 + 2 * n2], ADD)

            done_b = 0
            first_piece = 0
            sent16 = False
            for i in range(nb):
                tree(i)
                if not sent16 and S1 and cuts[i + 1] >= S1:
                    # p=1 class results ship as raw fp16 partials
                    nc.sync.dma_start(out16[:], r3[:, 0:S1])
                    sent16 = True
                if i == nb - 2 and Ft:
                    tail_trees()
                    nc.sync.dma_start(outd[:, smid:Sob], ob[:, smid:Sob])
                gdone = cuts[i + 1]
                while done_b < len(bspan):
                    g0, nslot, p, s0 = bspan[done_b]
                    if nslot and g0 + nslot * p > gdone:
                        break
                    if nslot:
                        body_l2(g0, nslot, p, s0)
                    done_b += 1
                if i == nb - 2 and done_b:
                    # overlap most of the body output with the last chunks
                    sdone = bspan[done_b - 1][3] + bspan[done_b - 1][1]
                    nc.sync.dma_start(outd[:, 0:sdone], ob[:, 0:sdone])
                    first_piece = sdone
            while done_b < len(bspan):
                g0, nslot, p, s0 = bspan[done_b]
                if nslot:
                    body_l2(g0, nslot, p, s0)
                done_b += 1

        nc.sync.dma_start(outd[:, first_piece:smid], ob[:, first_piece:smid])
    nc.finalize()
    return nc


# ---------------- entry point ----------------
def kernel(image_feature, post_trans, post_rots, intrinsics, extrinsics,
           frustum, bev_res, bev_start_pos):
    from concourse.bass_utils import run_bass_kernel_spmd
    import os

    rank, mask = _compute_ranks(frustum, post_trans, post_rots, intrinsics,
                                extrinsics, bev_res, bev_start_pos)
    feats16 = np.asarray(image_feature, np.float32).reshape(
        B * NP_SAMPLE, C).astype(np.float16)

    plan = _plan(rank, mask)
    orders, bbins, Sb, tbins, St, Fb, Ft, Stot, cuts = plan

    in_maps = []
    out_maps = []
    for core in range(NCORES):
        im, om = _build_core_inputs(core, plan, feats16)
        in_maps.append(im)
        out_maps.append(om)

    key = (tuple(Sb), tuple(St), Fb, Ft, Stot, cuts)
    if key not in _compiled:
        _compiled[key] = _build_kernel(Sb, St, Fb, Ft, Stot, cuts)
    nc = _compiled[key]

    trace = bool(int(os.environ.get("BEV_TRACE", "0")))
    res = run_bass_kernel_spmd(nc, in_maps, core_ids=list(range(NCORES)),
                               trace=trace,
                               trace_cores=[0] if trace else None)
    if trace and res.exec_time_ns is not None:
        print(f"HW exec time: {res.exec_time_ns} ns")
        kernel.last_exec_time_ns = res.exec_time_ns
        kernel.last_results = res

    grid = np.zeros((B, NBINS, C), np.float32)
    for core in range(NCORES):
        o16 = res.results[core]["out16"].astype(np.float32)
        o = np.concatenate([o16[:, :Sb[0]], res.results[core]["out"]], axis=1)
        out_sample, out_rank = out_maps[core]
        for h in range(2):
            # body and tail slot regions separately: within each, the
            # (sample, rank) pairs are unique, so fancy += is safe
            sel = np.nonzero(out_rank[h] >= 0)[0]
            np.add.at(grid, (out_sample[h][sel], out_rank[h][sel]),
                      o[h * C:(h + 1) * C, sel].T)
    return np.ascontiguousarray(
        grid.reshape(B, X, Y, C).transpose(0, 3, 1, 2))


# revision 28
# speedup vs baseline: 1.1543x; 1.1543x over previous
"""BEV voxel-pooling (segment_reduce) kernel for 8 Trainium2 NeuronCores.

Strategy
--------
The BEV pooling is a segment-sum of 346k points x 64 channels into ~12k
occupied bins (mean run length ~29).  The whole reduction runs on the
DVE with channels on the partition axis:

Host (numpy, driven only by the small geometry inputs):
  * compute each point's BEV rank (bin id) exactly as the reference
  * segments = runs of equal rank in the sorted point stream
  * split each segment into a BODY (groups of 8 points; a remainder of
    5-7 points becomes one extra zero-padded group) and a TAIL
    (remainder 1-4 points, padded to a power-of-2 class 2/4)
  * bucket bodies by padded group count; deal every class round-robin
    across 16 bins (8 cores x 2 partition-halves) so all cores get an
    identical layout and balanced points
  * per core, build a [128, F] fp16 table: partitions 0-63 hold the 64
    channels of the core's "half A" point stream, 64-127 of "half B".
    Within each body DMA chunk (and each tail class region) points are
    stored in PLANE layout: plane k holds the k-th point of every
    group, so every tree round below is a flat contiguous add (2x DVE)

Device (per core, one SPMD Bass/Tile program, DVE only):
  * DMA the table in ramped chunks, all on the sync queue (a single
    FIFO keeps all 16 DMA engines busy and avoids cross-queue races)
  * per body chunk: 3 flat tensor_tensor adds (halves of planes) fold
    8 points -> 1 group partial (fp16, 2x DVE mode)
  * per body class, one strided tensor_reduce (inner = padded group
    count) sums each segment's partials into the f32 output
  * tail classes are power-of-2 plane trees; the last round writes the
    f32 output columns directly
  * the f32 output [128, S] is DMA'd out in two pieces, the first
    overlapped with the last chunks' compute

Host gather: map each (core, half, class, slot) to its (sample, rank)
and accumulate all pieces into the (B, 40000, 64) grid with np.add.at
(a segment may contribute a body piece and a tail piece); reshape to
the reference layout (B, C, X, Y).
"""
import sys
sys.path.insert(0, '/opt/trn_rl_repo')

import numpy as np

# ---------------- problem constants (hardcoded per spec) ----------------
B, N, C = 2, 6, 64
H_IMG, W_IMG = 256, 704
DS = 16
DSH, DSW = H_IMG // DS, W_IMG // DS          # 16, 44
D0, D1 = 4, 45                                # depth bins -> D = 41
X, Y, Z = 200, 200, 1
NBINS = X * Y * Z                             # 40000
NP_SAMPLE = N * (D1 - D0) * DSH * DSW         # 173184
NCORES = 8
NBIN = NCORES * 2                             # (core, half) bins
G = 8                                         # body group size
BODY_CLASSES = (1, 2, 3, 4, 5, 6, 8, 10, 12, 16, 20, 24, 32, 48)
TAIL_CLASSES = (2, 4)                         # t>=5 folds into body q+=1
BODY_CUTS = (0.0, 0.025, 0.07, 0.15, 0.27, 0.43, 0.62, 0.81, 0.95, 1.0)

_compiled = {}


# ---------------- host geometry (matches reference numerics) ----------------
def _compute_ranks(frustum, post_trans, post_rots, intrinsics, extrinsics,
                   bev_res, bev_start_pos):
    frustum = np.asarray(frustum, np.float32)
    post_trans = np.asarray(post_trans, np.float32)
    post_rots = np.asarray(post_rots, np.float32)
    intrinsics = np.asarray(intrinsics, np.float32)
    extrinsics = np.asarray(extrinsics, np.float32)
    bev_res = np.asarray(bev_res, np.float32)
    bev_start_pos = np.asarray(bev_start_pos, np.float32)

    ext_inv = np.linalg.inv(extrinsics.astype(np.float64)).astype(np.float32)
    rot = ext_inv[..., :3, :3]
    trans = ext_inv[..., :3, 3]
    pts = frustum[None, None] - post_trans[:, :, None, None, None, :]
    pr_inv = np.linalg.inv(post_rots.astype(np.float64)).astype(np.float32)
    pts = np.einsum('bnij,bndhwj->bndhwi', pr_inv, pts).astype(np.float32)
    pts = np.concatenate([pts[..., :2] * pts[..., 2:3], pts[..., 2:3]], axis=-1)
    comb = (rot @ np.linalg.inv(intrinsics.astype(np.float64)).astype(np.float32)
            ).astype(np.float32)
    pts = np.einsum('bnij,bndhwj->bndhwi', comb, pts).astype(np.float32)
    geom = pts + trans[:, :, None, None, None, :]

    coords = (geom - (bev_start_pos - bev_res / 2.0)) / bev_res
    ci = coords.reshape(B, -1, 3).astype(np.int32)
    mask = ((ci[..., 0] >= 0) & (ci[..., 0] < X) &
            (ci[..., 1] >= 0) & (ci[..., 1] < Y) &
            (ci[..., 2] >= 0) & (ci[..., 2] < Z))
    rank = ci[..., 0] * (Y * Z) + ci[..., 1] * Z + ci[..., 2]
    return rank, mask


# ---------------- host planning ----------------
def _plan(rank, mask):
    bcl = np.asarray(BODY_CLASSES)
    tcl = np.asarray(TAIL_CLASSES)
    body_by_class = [[] for _ in BODY_CLASSES]
    tail_by_class = [[] for _ in TAIL_CLASSES]
    orders = []
    for b in range(B):
        m = mask[b]
        valid_idx = np.nonzero(m)[0]
        order = valid_idx[np.argsort(rank[b][valid_idx], kind='stable')]
        orders.append(order)
        rs = rank[b][order]
        new = np.r_[True, rs[1:] != rs[:-1]]
        starts = np.nonzero(new)[0]
        lens = np.diff(np.r_[starts, len(rs)])
        ranks = rs[starts]
        q, t = lens // G, lens % G
        # t >= 5 becomes an extra 8-padded body group (a second body piece,
        # class q=1) -- cheaper than a dedicated tail class
        big_t = t >= 5
        qi = np.searchsorted(bcl, q)
        ti = np.searchsorted(tcl, t)
        for j in range(len(starts)):
            if q[j] > 0:
                body_by_class[qi[j]].append(
                    (b, int(ranks[j]), int(starts[j]), int(q[j]) * G))
            if t[j] > 0:
                if big_t[j]:
                    body_by_class[0].append(
                        (b, int(ranks[j]), int(starts[j] + q[j] * G),
                         int(t[j])))
                else:
                    tail_by_class[ti[j]].append(
                        (b, int(ranks[j]), int(starts[j] + q[j] * G),
                         int(t[j])))

    def deal(by_class, rot):
        bins = [[[] for _ in by_class] for _ in range(NBIN)]
        S = []
        for c, segs in enumerate(by_class):
            segs.sort(key=lambda x: -x[3])
            for i, sg in enumerate(segs):
                bins[(rot + i) % NBIN][c].append(sg)
            S.append((len(segs) + NBIN - 1) // NBIN)
            rot = (rot + len(segs)) % NBIN
        return bins, S, rot

    bbins, Sb, rot = deal(body_by_class, 0)
    tbins, St, _ = deal(tail_by_class, rot)

    Fb = int(sum(Sb[c] * BODY_CLASSES[c] * G for c in range(len(BODY_CLASSES))))
    Ft = int(sum(St[c] * TAIL_CLASSES[c] for c in range(len(TAIL_CLASSES))))
    Stot = int(sum(Sb) + sum(St))

    # body chunk cuts at group granularity
    ngroups = Fb // G
    cuts = [int(round(f * ngroups)) for f in BODY_CUTS]
    cuts[-1] = ngroups
    return orders, bbins, Sb, tbins, St, Fb, Ft, Stot, tuple(cuts)


def _build_core_inputs(core, plan, feats16):
    orders, bbins, Sb, tbins, St, Fb, Ft, Stot, cuts = plan
    F = Fb + Ft
    ngroups = Fb // G
    table = np.zeros((128, F), np.float16)
    out_sample = np.zeros((2, Stot), np.int32)
    out_rank = np.full((2, Stot), -1, np.int32)
    for h in range(2):
        # group-major point index grid [ngroups, 8] (-1 = pad)
        gp = np.full((ngroups, G), -1, np.int64)
        g0 = 0
        slot = 0
        for c, p in enumerate(BODY_CLASSES):
            for s, (b, r, st, ln) in enumerate(bbins[2 * core + h][c]):
                rows = orders[b][st:st + ln] + b * NP_SAMPLE
                gbase = g0 + s * p
                full = np.full(((ln + G - 1) // G) * G, -1, np.int64)
                full[:ln] = rows
                gp[gbase:gbase + len(full) // G] = full.reshape(-1, G)
                out_sample[h, slot + s] = b
                out_rank[h, slot + s] = r
            g0 += Sb[c] * p
            slot += Sb[c]
        # plane layout per chunk
        idx = np.empty(F, np.int64)
        for i in range(len(cuts) - 1):
            ga, gb = cuts[i], cuts[i + 1]
            idx[ga * G:gb * G] = gp[ga:gb].T.reshape(-1)
        # tail region: per class, [S_t, t] -> planes [t, S_t]
        off = Fb
        for c, t in enumerate(TAIL_CLASSES):
            tp = np.full((St[c], t), -1, np.int64)
            for s, (b, r, st, ln) in enumerate(tbins[2 * core + h][c]):
                tp[s, :ln] = orders[b][st:st + ln] + b * NP_SAMPLE
                out_sample[h, slot + s] = b
                out_rank[h, slot + s] = r
            idx[off:off + St[c] * t] = tp.T.reshape(-1)
            off += St[c] * t
            slot += St[c]
        sel = idx >= 0
        tah = np.zeros((F, C), np.float16)
        tah[sel] = feats16[idx[sel]]
        table[h * C:(h + 1) * C] = tah.T
    return {"table": table}, (out_sample, out_rank)


# ---------------- device program ----------------
def _build_kernel(Sb, St, Fb, Ft, Stot, cuts):
    import concourse.bass as bass
    import concourse.bacc as bacc
    import concourse.mybir as mybir
    import concourse.tile as tile
    from contextlib import ExitStack

    F16 = mybir.dt.float16
    F32 = mybir.dt.float32
    ADD = mybir.AluOpType.add
    nc = bacc.Bacc()
    F = Fb + Ft
    S1 = Sb[0]                                 # p=1 class: r3 IS the result
    Sob = Stot - S1                            # f32 output columns
    table = nc.dram_tensor("table", [128, F], F16, kind="ExternalInput")
    outd = nc.dram_tensor("out", [128, Sob], F32, kind="ExternalOutput")
    out16 = nc.dram_tensor("out16", [128, max(S1, 1)], F16,
                           kind="ExternalOutput")

    nb = len(cuts) - 1
    # body class spans (classes >= 2; class p=1 ships straight from r3)
    bspan = []
    goff = S1
    soff = 0
    for c in range(1, len(BODY_CLASSES)):
        p = BODY_CLASSES[c]
        bspan.append((goff, Sb[c], p, soff))
        goff += Sb[c] * p
        soff += Sb[c]
    smid = soff                                # first tail output slot

    with tile.TileContext(nc) as tc, ExitStack() as ctx:
        pool = ctx.enter_context(tc.tile_pool(name="main", bufs=1))
        tbl = pool.tile([128, F], F16)
        r1 = pool.tile([128, Fb // 2], F16)
        r2 = pool.tile([128, Fb // 4], F16)
        r3 = pool.tile([128, Fb // 8], F16)
        t4h = pool.tile([128, max(St[1] * 2, 1)], F16)
        ob = pool.tile([128, Sob], F32)

        with nc.allow_low_precision(reason="fp16 tree partials; quantization "
                                    "error well under the 2e-2 gate"):
            # input DMAs all on the sync queue (keeps all 16 DMA engines on
            # one FIFO; chunk 0 small so compute starts early, tail region
            # second so its reduces fill the early pipeline)
            nc.sync.dma_start(tbl[:, 0:cuts[1] * G], table[:, 0:cuts[1] * G])
            for i in range(1, nb):
                a, b = cuts[i] * G, cuts[i + 1] * G
                nc.sync.dma_start(tbl[:, a:b], table[:, a:b])
                if i == nb - 3 and Ft:
                    nc.sync.dma_start(tbl[:, Fb:F], table[:, Fb:F])

            def tree(i):
                ga, gb = cuts[i], cuts[i + 1]
                g = gb - ga
                a = ga * G
                nc.vector.tensor_tensor(
                    r1[:, ga * 4:gb * 4],
                    tbl[:, a:a + 4 * g], tbl[:, a + 4 * g:a + 8 * g], ADD)
                nc.vector.tensor_tensor(
                    r2[:, ga * 2:gb * 2],
                    r1[:, ga * 4:ga * 4 + 2 * g],
                    r1[:, ga * 4 + 2 * g:gb * 4], ADD)
                nc.vector.tensor_tensor(
                    r3[:, ga:gb],
                    r2[:, ga * 2:ga * 2 + g], r2[:, ga * 2 + g:gb * 2], ADD)

            def body_l2(g0, nslot, p, s0):
                src = r3[:, g0:g0 + nslot * p].rearrange("p (s q) -> p s q", q=p)
                nc.vector.tensor_reduce(ob[:, s0:s0 + nslot], src,
                                        axis=mybir.AxisListType.X, op=ADD)

            def tail_trees():
                # class order in table/slots: t=2, t=4 (plane layout)
                n2, n4 = St[0], St[1]
                o2 = Fb
                o4 = o2 + n2 * 2
                s4 = smid + n2
                if n4:
                    nc.vector.tensor_tensor(
                        t4h[:, :n4 * 2],
                        tbl[:, o4:o4 + 2 * n4], tbl[:, o4 + 2 * n4:o4 + 4 * n4],
                        ADD)
                    nc.vector.tensor_tensor(
                        ob[:, s4:s4 + n4],
                        t4h[:, :n4], t4h[:, n4:2 * n4], ADD)
                if n2:
                    nc.vector.tensor_tensor(
                        ob[:, smid:smid + n2],
                        tbl[:, o2:o2 + n2], tbl[:, o2 + n2:o2 + 2 * n2], ADD)

            done_b = 0
            first_piece = 0
            sent16 = False
            for i in range(nb):
                tree(i)
                if not sent16 and S1 and cuts[i + 1] >= S1:
                    # p=1 class results ship as raw fp16 partials
                    nc.sync.dma_start(out16[:], r3[:, 0:S1])
                    sent16 = True
                if i == nb - 2 and Ft:
                    tail_trees()
                    nc.sync.dma_start(outd[:, smid:Sob], ob[:, smid:Sob])
                gdone = cuts[i + 1]
                while done_b < len(bspan):
                    g0, nslot, p, s0 = bspan[done_b]
                    if nslot and g0 + nslot * p > gdone:
                        break
                    if nslot:
                        body_l2(g0, nslot, p, s0)
                    done_b += 1
                if i == nb - 2 and done_b:
                    # overlap most of the body output with the last chunks
                    sdone = bspan[done_b - 1][3] + bspan[done_b - 1][1]
                    nc.sync.dma_start(outd[:, 0:sdone], ob[:, 0:sdone])
                    first_piece = sdone
            while done_b < len(bspan):
                g0, nslot, p, s0 = bspan[done_b]
                if nslot:
                    body_l2(g0, nslot, p, s0)
                done_b += 1

        nc.sync.dma_start(outd[:, first_piece:smid], ob[:, first_piece:smid])
    nc.finalize()
    return nc


# ---------------- entry point ----------------
def kernel(image_feature, post_trans, post_rots, intrinsics, extrinsics,
           frustum, bev_res, bev_start_pos):
    from concourse.bass_utils import run_bass_kernel_spmd
    import os

    rank, mask = _compute_ranks(frustum, post_trans, post_rots, intrinsics,
                                extrinsics, bev_res, bev_start_pos)
    feats16 = np.asarray(image_feature, np.float32).reshape(
        B * NP_SAMPLE, C).astype(np.float16)

    plan = _plan(rank, mask)
    orders, bbins, Sb, tbins, St, Fb, Ft, Stot, cuts = plan

    in_maps = []
    out_maps = []
    for core in range(NCORES):
        im, om = _build_core_inputs(core, plan, feats16)
        in_maps.append(im)
        out_maps.append(om)

    key = (tuple(Sb), tuple(St), Fb, Ft, Stot, cuts)
    if key not in _compiled:
        _compiled[key] = _build_kernel(Sb, St, Fb, Ft, Stot, cuts)
    nc = _compiled[key]

    trace = bool(int(os.environ.get("BEV_TRACE", "0")))
    res = run_bass_kernel_spmd(nc, in_maps, core_ids=list(range(NCORES)),
                               trace=trace,
                               trace_cores=[0] if trace else None)
    if trace and res.exec_time_ns is not None:
        print(f"HW exec time: {res.exec_time_ns} ns")
        kernel.last_exec_time_ns = res.exec_time_ns
        kernel.last_results = res

    grid = np.zeros((B, NBINS, C), np.float32)
    for core in range(NCORES):
        o16 = res.results[core]["out16"].astype(np.float32)
        o = np.concatenate([o16[:, :Sb[0]], res.results[core]["out"]], axis=1)
        out_sample, out_rank = out_maps[core]
        for h in range(2):
            # body and tail slot regions separately: within each, the
            # (sample, rank) pairs are unique, so fancy += is safe
            sel = np.nonzero(out_rank[h] >= 0)[0]
            np.add.at(grid, (out_sample[h][sel], out_rank[h][sel]),
                      o[h * C:(h + 1) * C, sel].T)
    return np.ascontiguousarray(
        grid.reshape(B, X, Y, C).transpose(0, 3, 1, 2))
